# revision 14
# baseline (speedup 1.0000x reference)
"""Locally-connected layer (3x3, stride 1) on 8 Trainium2 NeuronCores.

Shapes (hardcoded):
  x      [B=32, C=96, H=32, W=32]  fp32
  weight [P=900, O=96, K=864]      fp32   (K = C*3*3, channel-major (c,kh,kw))
  bias   [P=900, O=96]             fp32
  out    [B=32, O=96, 30, 30]      fp32

Strategy:
  - Shard the 30x30 patch grid by output rows, padded to 32 rows -> 4 rows
    (120 patches) per core.  One SPMD program on all 8 cores.
  - Host casts x to bf16 and weight to float8_e3m4 scaled by 256 (the host
    divides the output by 256 afterwards).  Measured rel-err 1.23e-2 vs the
    2e-2 budget.  The weight stream (9.95 MB/core, used exactly once) is the
    bandwidth bottleneck, so 1-byte weights halve the roofline vs bf16.
  - Per patch, contract K=864 as 9 accumulating matmuls of K=C=96:
    out[b,o] += x[:, i+di, j+dj, b].T @ W[p, dd][:, o].
  - Stationary (lhsT) = x columns [96c, 32b] read in place from an SBUF-
    resident x slice laid out [c, h, w, b]; moving (rhs) = per-patch weight
    [96c, 96o].  Groups of 4 (or 2/3) adjacent patches are col-tiled onto
    the 128-wide PE array via tile_position=(0, 32u).
  - Weights stream from HBM in 15-patch half-row chunks alternating across
    the two HWDGE queues (sync/scalar) so per-chunk completion latencies
    overlap, multi-buffered; the last row uses 10-patch chunks so the
    PE+store tail after the final DMA is short.
  - Output is written bf16 per chunk (host upcasts, descales, adds bias);
    the final chunk's store rides the low-latency sync ring.
"""

import numpy as np

B, C, O, H, W = 32, 96, 96, 32, 32
OH = OW = 30
NCORES = 8
ROWS_PER_CORE = 4            # padded 32 output rows / 8 cores
P_CORE = ROWS_PER_CORE * OW  # 120 patches per core
XROWS = ROWS_PER_CORE + 2    # input rows needed per core (halo)
WSCALE = 256.0               # e3m4 pre-scale (|w*256| < 15.5 = e3m4 max)

LAST_RESULT = None           # BassKernelResults of the most recent run
_NC_CACHE = {}
KERNEL_KW = {}               # _build_bass kwargs for the kernel() path


def _chunk_groups(cp):
    """Split a chunk of cp consecutive patches into col-tile groups of <=4."""
    groups, j = [], 0
    while j < cp:
        g = min(4, cp - j)
        if cp - j == 5:      # avoid a trailing group of 1
            g = 3
        groups.append((j, g))
        j += g
    return groups


def _schedule(tail=10, split30=False):
    """Chunk schedule: list of (li, j0, cp).  Rows 0..2 as one chunk each
    (or two 15-patch halves when split30, so two HWDGE queues can overlap
    per-chunk completion latencies); the last row split into `tail`-sized
    chunks so the post-DMA tail is short."""
    chunks = []
    for li in range(ROWS_PER_CORE - 1):
        if split30:
            chunks += [(li, 0, 15), (li, 15, 15)]
        else:
            chunks.append((li, 0, OW))
    assert OW % tail == 0
    for j0 in range(0, OW, tail):
        chunks.append((ROWS_PER_CORE - 1, j0, tail))
    return chunks


def _out_layout(tail=10, split30=False):
    """Per-chunk group offsets in the ot tile: returns (chunks, group_off)
    where group_off[ci] is the first group index of chunk ci."""
    chunks = _schedule(tail, split30)
    off, group_off = 0, []
    for (li, j0, cp) in chunks:
        group_off.append(off)
        off += len(_chunk_groups(cp))
    return chunks, group_off, off


def _build_bass(reps=1, with_wdma=True, with_mm=True, with_out=True,
                tail=10, wbufs=8, alt_ring=True, wdt="e3m4", fillers=0,
                split30=True):
    import concourse.bass as bass
    import concourse.mybir as mybir
    import concourse.tile as tile
    from concourse import bacc

    chunks, group_off, n_groups = _out_layout(tail, split30)
    otw = n_groups * O
    cpmax = max(cp for _, _, cp in chunks)
    ppbufs = 7 if fillers else 8         # leave one PSUM bank for the scratch

    f32 = mybir.dt.float32
    bf16 = mybir.dt.bfloat16
    w_dt = {"bf16": bf16, "e3m4": mybir.dt.float8e3}[wdt]
    nc = bacc.Bacc("TRN2", target_bir_lowering=False, debug=False,
                   num_devices=NCORES)
    xsd = nc.dram_tensor("xs", [C, XROWS, W, B], bf16, kind="ExternalInput")
    wsd = nc.dram_tensor("ws", [C, P_CORE, 9, O], w_dt, kind="ExternalInput")
    od = nc.dram_tensor("out", [128, otw], bf16, kind="ExternalOutput")

    with tile.TileContext(nc) as tc:
        with (
            tc.tile_pool(name="xp", bufs=1) as xp,
            tc.tile_pool(name="wp", bufs=wbufs) as wp,
            tc.tile_pool(name="op", bufs=1) as op,
            tc.tile_pool(name="pp", bufs=ppbufs, space=bass.MemorySpace.PSUM) as pp,
            tc.tile_pool(name="fp", bufs=1, space=bass.MemorySpace.PSUM) as fp,
        ):
            xt = xp.tile([C, XROWS, W, B], bf16)
            # x rides the SWDGE ring so the HWDGE ring(s) are dedicated to
            # the weight stream (the critical path)
            nc.gpsimd.dma_start(xt[:], xsd[:])
            ot = op.tile([128, otw], bf16)

            wt_fixed = None
            if not with_wdma:
                # mm-only probe: one persistent weight tile, loaded once
                wt_fixed = xp.tile([C, cpmax, 9, O], w_dt)
                nc.sync.dma_start(wt_fixed[:], wsd[:, 0:cpmax, :, :])
            if not with_mm and with_out:
                nc.vector.memset(ot[:], 0.0)

            for _rep in range(reps):
                p0 = 0
                for ci, (li, j0, cp) in enumerate(chunks):
                    last = ci == len(chunks) - 1
                    if with_wdma:
                        wt = wp.tile([C, cp, 9, O], w_dt)
                        src = wsd[:, p0:p0 + cp, :, :]
                        if alt_ring and ci % 2 == 1:
                            nc.scalar.dma_start(wt[:], src)
                        else:
                            nc.sync.dma_start(wt[:], src)
                    else:
                        wt = wt_fixed
                    if with_mm:
                        for gi, (jo, gsz) in enumerate(_chunk_groups(cp)):
                            jg = j0 + jo
                            ps = pp.tile([128, O], f32)
                            for dd in range(9):
                                di, dj = dd // 3, dd % 3
                                for u in range(gsz):
                                    nc.tensor.matmul(
                                        ps[32 * u:32 * (u + 1), :],
                                        xt[:, li + di, jg + u + dj, :],
                                        wt[:, jo + u, dd, :],
                                        start=(dd == 0),
                                        stop=(dd == 8),
                                        tile_position=(0, 32 * u),
                                    )
                            g = group_off[ci] + gi
                            nc.vector.tensor_copy(
                                ot[0:32 * gsz, g * O:(g + 1) * O],
                                ps[0:32 * gsz, :])
                    if with_mm and fillers and cp == OW and not last:
                        # keep the PE HAM activity window open across the
                        # DMA wait for the next chunk: dummy matmuls into a
                        # scratch bank, data-dependent on this chunk's ot
                        # region so they cannot be hoisted earlier
                        ge = (group_off[ci] + len(_chunk_groups(cp))) * O
                        sc = fp.tile([32, 512], f32)
                        for _k in range(fillers):
                            nc.tensor.matmul(
                                sc[:, :],
                                ot[0:C, ge - 32:ge],
                                ot[0:C, ge - 512:ge],
                                start=True, stop=True,
                            )
                    if with_out:
                        g0, g1 = group_off[ci], group_off[ci] + len(
                            _chunk_groups(cp))
                        dst = od[:, g0 * O:g1 * O]
                        srco = ot[:, g0 * O:g1 * O]
                        if last:
                            nc.sync.dma_start(dst, srco)
                        else:
                            nc.gpsimd.dma_start(dst, srco)
                    p0 += cp
    nc.compile()
    return nc


def _get_nc():
    key = tuple(sorted(KERNEL_KW.items()))
    if key not in _NC_CACHE:
        _NC_CACHE[key] = _build_bass(**KERNEL_KW)
    return _NC_CACHE[key]


def _prep_in_maps(x, weight, wdt="e3m4"):
    import ml_dtypes
    bf16 = ml_dtypes.bfloat16

    if wdt == "e3m4":
        wq = (weight.astype(np.float32) * WSCALE).astype(ml_dtypes.float8_e3m4)
    else:
        wq = weight.astype(bf16)
    # weight [900, O, C*3*3] -> [C, P_pad=960, dd, O]
    w5 = wq.reshape(OH * OW, O, C, 3, 3)
    wt = w5.transpose(2, 0, 3, 4, 1).reshape(C, OH * OW, 9, O)
    wpad = np.zeros((C, NCORES * P_CORE, 9, O), dtype=wq.dtype)
    wpad[:, :OH * OW] = wt

    # x [B, C, H, W] -> bf16 [C, H_pad=34, W, B]
    xt = x.astype(bf16).transpose(1, 2, 3, 0)
    xpad = np.zeros((C, H + 2, W, B), dtype=bf16)
    xpad[:, :H] = xt

    in_maps = []
    for c in range(NCORES):
        in_maps.append({
            "xs": np.ascontiguousarray(
                xpad[:, ROWS_PER_CORE * c:ROWS_PER_CORE * c + XROWS]),
            "ws": np.ascontiguousarray(
                wpad[:, P_CORE * c:P_CORE * (c + 1)]),
        })
    return in_maps


def kernel(x, weight, bias):
    global LAST_RESULT
    from concourse.bass_utils import run_bass_kernel_spmd

    x = np.asarray(x, dtype=np.float32)
    weight = np.asarray(weight, dtype=np.float32)
    bias = np.asarray(bias, dtype=np.float32)

    wdt = KERNEL_KW.get("wdt", "e3m4")
    in_maps = _prep_in_maps(x, weight, wdt=wdt)
    nc = _get_nc()
    LAST_RESULT = run_bass_kernel_spmd(
        nc, in_maps, core_ids=list(range(NCORES)), trace=False)

    # ---- gather: per-core [128, n_groups*96] -> full [B, O, 30, 30] ----
    tail = KERNEL_KW.get("tail", 10)
    chunks, group_off, n_groups = _out_layout(
        tail, KERNEL_KW.get("split30", True))
    out = np.zeros((B, O, OH, OW), dtype=np.float32)
    for c in range(NCORES):
        oc = LAST_RESULT.results[c]["out"].astype(np.float32)
        if wdt == "e3m4":
            oc = oc / WSCALE
        oc = oc.reshape(4, 32, n_groups, O)
        for ci, (li, j0, cp) in enumerate(chunks):
            i = ROWS_PER_CORE * c + li
            if i >= OH:
                continue
            for gi, (jo, gsz) in enumerate(_chunk_groups(cp)):
                g = group_off[ci] + gi
                blk = oc[:gsz, :, g, :]            # [u, b, o]
                out[:, :, i, j0 + jo:j0 + jo + gsz] = blk.transpose(1, 2, 0)
    out += bias.reshape(OH, OW, O).transpose(2, 0, 1)[None]
    return out


# revision 16
# speedup vs baseline: 1.1610x; 1.1610x over previous
"""Locally-connected layer (3x3, stride 1) on 8 Trainium2 NeuronCores.

Shapes (hardcoded):
  x      [B=32, C=96, H=32, W=32]  fp32
  weight [P=900, O=96, K=864]      fp32   (K = C*3*3, channel-major (c,kh,kw))
  bias   [P=900, O=96]             fp32
  out    [B=32, O=96, 30, 30]      fp32

Strategy:
  - Shard the 30x30 patch grid by output rows, padded to 32 rows -> 4 rows
    (120 patches) per core.  One SPMD program on all 8 cores.
  - Host casts x to bf16 and weight to float8_e3m4 scaled by 256 (the host
    divides the output by 256 afterwards).  Measured rel-err 1.23e-2 vs the
    2e-2 budget.  The weight stream (9.95 MB/core, used exactly once) is the
    bandwidth bottleneck, so 1-byte weights halve the roofline vs bf16.
  - Per patch, contract K=864 as 9 accumulating matmuls of K=C=96:
    out[b,o] += x[:, i+di, j+dj, b].T @ W[p, dd][:, o].
  - Stationary (lhsT) = x columns [96c, 32b] read in place from an SBUF-
    resident x slice laid out [c, h, w, b]; moving (rhs) = per-patch weight
    [96c, 96o].  Groups of 4 (or 2/3) adjacent patches are col-tiled onto
    the 128-wide PE array via tile_position=(0, 32u).
  - Weights stream from HBM in 10-patch third-row chunks alternating
    across the two HWDGE queues (sync/scalar) so per-chunk completion
    latencies overlap, 8-deep buffered; the short chunks also keep the
    PE+store tail after the final DMA small.
  - Output is written bf16 per chunk (host upcasts, descales, adds bias);
    the final chunk's store rides the low-latency sync ring.
"""

import numpy as np

B, C, O, H, W = 32, 96, 96, 32, 32
OH = OW = 30
NCORES = 8
ROWS_PER_CORE = 4            # padded 32 output rows / 8 cores
P_CORE = ROWS_PER_CORE * OW  # 120 patches per core
XROWS = ROWS_PER_CORE + 2    # input rows needed per core (halo)
WSCALE = 256.0               # e3m4 pre-scale (|w*256| < 15.5 = e3m4 max)

LAST_RESULT = None           # BassKernelResults of the most recent run
_NC_CACHE = {}
KERNEL_KW = {}               # _build_bass kwargs for the kernel() path


def _chunk_groups(cp):
    """Split a chunk of cp consecutive patches into col-tile groups of <=4."""
    groups, j = [], 0
    while j < cp:
        g = min(4, cp - j)
        if cp - j == 5:      # avoid a trailing group of 1
            g = 3
        groups.append((j, g))
        j += g
    return groups


def _schedule(tail=10, split30=False):
    """Chunk schedule: list of (li, j0, cp).  Rows 0..2 as one chunk each
    (or two 15-patch halves when split30, so two HWDGE queues can overlap
    per-chunk completion latencies); the last row split into `tail`-sized
    chunks so the post-DMA tail is short."""
    chunks = []
    for li in range(ROWS_PER_CORE - 1):
        if split30 == "ten":
            chunks += [(li, 0, 10), (li, 10, 10), (li, 20, 10)]
        elif split30:
            chunks += [(li, 0, 15), (li, 15, 15)]
        else:
            chunks.append((li, 0, OW))
    assert OW % tail == 0
    for j0 in range(0, OW, tail):
        chunks.append((ROWS_PER_CORE - 1, j0, tail))
    return chunks


def _out_layout(tail=10, split30=False):
    """Per-chunk group offsets in the ot tile: returns (chunks, group_off)
    where group_off[ci] is the first group index of chunk ci."""
    chunks = _schedule(tail, split30)
    off, group_off = 0, []
    for (li, j0, cp) in chunks:
        group_off.append(off)
        off += len(_chunk_groups(cp))
    return chunks, group_off, off


def _build_bass(reps=1, with_wdma=True, with_mm=True, with_out=True,
                tail=10, wbufs=8, alt_ring=True, wdt="e3m4", fillers=0,
                split30="ten", rot_ring=False):
    import concourse.bass as bass
    import concourse.mybir as mybir
    import concourse.tile as tile
    from concourse import bacc

    chunks, group_off, n_groups = _out_layout(tail, split30)
    otw = n_groups * O
    cpmax = max(cp for _, _, cp in chunks)
    ppbufs = 7 if fillers else 8         # leave one PSUM bank for the scratch

    f32 = mybir.dt.float32
    bf16 = mybir.dt.bfloat16
    w_dt = {"bf16": bf16, "e3m4": mybir.dt.float8e3}[wdt]
    nc = bacc.Bacc("TRN2", target_bir_lowering=False, debug=False,
                   num_devices=NCORES)
    xsd = nc.dram_tensor("xs", [C, XROWS, W, B], bf16, kind="ExternalInput")
    wsd = nc.dram_tensor("ws", [C, P_CORE, 9, O], w_dt, kind="ExternalInput")
    od = nc.dram_tensor("out", [128, otw], bf16, kind="ExternalOutput")

    with tile.TileContext(nc) as tc:
        with (
            tc.tile_pool(name="xp", bufs=1) as xp,
            tc.tile_pool(name="wp", bufs=wbufs) as wp,
            tc.tile_pool(name="op", bufs=1) as op,
            tc.tile_pool(name="pp", bufs=ppbufs, space=bass.MemorySpace.PSUM) as pp,
            tc.tile_pool(name="fp", bufs=1, space=bass.MemorySpace.PSUM) as fp,
        ):
            xt = xp.tile([C, XROWS, W, B], bf16)
            # x rides the SWDGE ring so the HWDGE ring(s) are dedicated to
            # the weight stream (the critical path)
            nc.gpsimd.dma_start(xt[:], xsd[:])
            ot = op.tile([128, otw], bf16)

            wt_fixed = None
            if not with_wdma:
                # mm-only probe: one persistent weight tile, loaded once
                wt_fixed = xp.tile([C, cpmax, 9, O], w_dt)
                nc.sync.dma_start(wt_fixed[:], wsd[:, 0:cpmax, :, :])
            if not with_mm and with_out:
                nc.vector.memset(ot[:], 0.0)

            for _rep in range(reps):
                p0 = 0
                for ci, (li, j0, cp) in enumerate(chunks):
                    last = ci == len(chunks) - 1
                    if with_wdma:
                        wt = wp.tile([C, cp, 9, O], w_dt)
                        src = wsd[:, p0:p0 + cp, :, :]
                        if rot_ring:
                            eng = (nc.sync, nc.scalar,
                                   nc.gpsimd)[ci % 3]
                            eng.dma_start(wt[:], src)
                        elif alt_ring and ci % 2 == 1:
                            nc.scalar.dma_start(wt[:], src)
                        else:
                            nc.sync.dma_start(wt[:], src)
                    else:
                        wt = wt_fixed
                    if with_mm:
                        for gi, (jo, gsz) in enumerate(_chunk_groups(cp)):
                            jg = j0 + jo
                            ps = pp.tile([128, O], f32)
                            for dd in range(9):
                                di, dj = dd // 3, dd % 3
                                for u in range(gsz):
                                    nc.tensor.matmul(
                                        ps[32 * u:32 * (u + 1), :],
                                        xt[:, li + di, jg + u + dj, :],
                                        wt[:, jo + u, dd, :],
                                        start=(dd == 0),
                                        stop=(dd == 8),
                                        tile_position=(0, 32 * u),
                                    )
                            g = group_off[ci] + gi
                            nc.vector.tensor_copy(
                                ot[0:32 * gsz, g * O:(g + 1) * O],
                                ps[0:32 * gsz, :])
                    if with_mm and fillers and cp == OW and not last:
                        # keep the PE HAM activity window open across the
                        # DMA wait for the next chunk: dummy matmuls into a
                        # scratch bank, data-dependent on this chunk's ot
                        # region so they cannot be hoisted earlier
                        ge = (group_off[ci] + len(_chunk_groups(cp))) * O
                        sc = fp.tile([32, 512], f32)
                        for _k in range(fillers):
                            nc.tensor.matmul(
                                sc[:, :],
                                ot[0:C, ge - 32:ge],
                                ot[0:C, ge - 512:ge],
                                start=True, stop=True,
                            )
                    if with_out:
                        g0, g1 = group_off[ci], group_off[ci] + len(
                            _chunk_groups(cp))
                        dst = od[:, g0 * O:g1 * O]
                        srco = ot[:, g0 * O:g1 * O]
                        if last:
                            nc.sync.dma_start(dst, srco)
                        else:
                            nc.gpsimd.dma_start(dst, srco)
                    p0 += cp
    nc.compile()
    return nc


def _get_nc():
    key = tuple(sorted(KERNEL_KW.items()))
    if key not in _NC_CACHE:
        _NC_CACHE[key] = _build_bass(**KERNEL_KW)
    return _NC_CACHE[key]


def _prep_in_maps(x, weight, wdt="e3m4"):
    import ml_dtypes
    bf16 = ml_dtypes.bfloat16

    if wdt == "e3m4":
        wq = (weight.astype(np.float32) * WSCALE).astype(ml_dtypes.float8_e3m4)
    else:
        wq = weight.astype(bf16)
    # weight [900, O, C*3*3] -> [C, P_pad=960, dd, O]
    w5 = wq.reshape(OH * OW, O, C, 3, 3)
    wt = w5.transpose(2, 0, 3, 4, 1).reshape(C, OH * OW, 9, O)
    wpad = np.zeros((C, NCORES * P_CORE, 9, O), dtype=wq.dtype)
    wpad[:, :OH * OW] = wt

    # x [B, C, H, W] -> bf16 [C, H_pad=34, W, B]
    xt = x.astype(bf16).transpose(1, 2, 3, 0)
    xpad = np.zeros((C, H + 2, W, B), dtype=bf16)
    xpad[:, :H] = xt

    in_maps = []
    for c in range(NCORES):
        in_maps.append({
            "xs": np.ascontiguousarray(
                xpad[:, ROWS_PER_CORE * c:ROWS_PER_CORE * c + XROWS]),
            "ws": np.ascontiguousarray(
                wpad[:, P_CORE * c:P_CORE * (c + 1)]),
        })
    return in_maps


def kernel(x, weight, bias):
    global LAST_RESULT
    from concourse.bass_utils import run_bass_kernel_spmd

    x = np.asarray(x, dtype=np.float32)
    weight = np.asarray(weight, dtype=np.float32)
    bias = np.asarray(bias, dtype=np.float32)

    wdt = KERNEL_KW.get("wdt", "e3m4")
    in_maps = _prep_in_maps(x, weight, wdt=wdt)
    nc = _get_nc()
    LAST_RESULT = run_bass_kernel_spmd(
        nc, in_maps, core_ids=list(range(NCORES)), trace=False)

    # ---- gather: per-core [128, n_groups*96] -> full [B, O, 30, 30] ----
    tail = KERNEL_KW.get("tail", 10)
    chunks, group_off, n_groups = _out_layout(
        tail, KERNEL_KW.get("split30", "ten"))
    out = np.zeros((B, O, OH, OW), dtype=np.float32)
    for c in range(NCORES):
        oc = LAST_RESULT.results[c]["out"].astype(np.float32)
        if wdt == "e3m4":
            oc = oc / WSCALE
        oc = oc.reshape(4, 32, n_groups, O)
        for ci, (li, j0, cp) in enumerate(chunks):
            i = ROWS_PER_CORE * c + li
            if i >= OH:
                continue
            for gi, (jo, gsz) in enumerate(_chunk_groups(cp)):
                g = group_off[ci] + gi
                blk = oc[:gsz, :, g, :]            # [u, b, o]
                out[:, :, i, j0 + jo:j0 + jo + gsz] = blk.transpose(1, 2, 0)
    out += bias.reshape(OH, OW, O).transpose(2, 0, 1)[None]
    return out


# revision 18
# speedup vs baseline: 1.2437x; 1.0713x over previous
"""Locally-connected layer (3x3, stride 1) on 8 Trainium2 NeuronCores.

Shapes (hardcoded):
  x      [B=32, C=96, H=32, W=32]  fp32
  weight [P=900, O=96, K=864]      fp32   (K = C*3*3, channel-major (c,kh,kw))
  bias   [P=900, O=96]             fp32
  out    [B=32, O=96, 30, 30]      fp32

Strategy:
  - Shard the 30x30 patch grid by output rows, padded to 32 rows -> 4 rows
    (120 patches) per core.  One SPMD program on all 8 cores.
  - Host casts x to bf16 and weight to float8_e3m4 scaled by 256 (the host
    divides the output by 256 afterwards).  Measured rel-err 1.23e-2 vs the
    2e-2 budget.  The weight stream (9.95 MB/core, used exactly once) is the
    bandwidth bottleneck, so 1-byte weights halve the roofline vs bf16.
  - Per patch, contract K=864 as 9 accumulating matmuls of K=C=96:
    out[b,o] += x[:, i+di, j+dj, b].T @ W[p, dd][:, o].
  - Stationary (lhsT) = x columns [96c, 32b] read in place from an SBUF-
    resident x slice laid out [c, h, w, b]; moving (rhs) = per-patch weight
    [96c, 96o].  Groups of 4 (or 2/3) adjacent patches are col-tiled onto
    the 128-wide PE array via tile_position=(0, 32u).
  - Weights stream from HBM in 5-patch chunks (24 per core) alternating
    across the two HWDGE queues (sync/scalar), 8-deep buffered: many small
    transfers fully pipeline the per-DMA fixed costs across the queues and
    keep the PE+store tail after the final DMA tiny.
  - Output is written bf16 per chunk (host upcasts, descales, adds bias);
    the final chunk's store rides the low-latency sync ring.
"""

import numpy as np

B, C, O, H, W = 32, 96, 96, 32, 32
OH = OW = 30
NCORES = 8
ROWS_PER_CORE = 4            # padded 32 output rows / 8 cores
P_CORE = ROWS_PER_CORE * OW  # 120 patches per core
XROWS = ROWS_PER_CORE + 2    # input rows needed per core (halo)
WSCALE = 256.0               # e3m4 pre-scale (|w*256| < 15.5 = e3m4 max)

LAST_RESULT = None           # BassKernelResults of the most recent run
_NC_CACHE = {}
KERNEL_KW = {}               # _build_bass kwargs for the kernel() path


def _chunk_groups(cp):
    """Split a chunk of cp consecutive patches into col-tile groups of <=4."""
    groups, j = [], 0
    while j < cp:
        g = min(4, cp - j)
        if cp - j == 5:      # avoid a trailing group of 1
            g = 3
        groups.append((j, g))
        j += g
    return groups


def _schedule(tail=10, split30=False):
    """Chunk schedule: list of (li, j0, cp).  Rows 0..2 as one chunk each
    (or two 15-patch halves when split30, so two HWDGE queues can overlap
    per-chunk completion latencies); the last row split into `tail`-sized
    chunks so the post-DMA tail is short."""
    chunks = []
    if split30 == "ten":
        split30 = 10
    elif split30 is True:
        split30 = 15
    for li in range(ROWS_PER_CORE - 1):
        if split30:
            assert OW % split30 == 0
            chunks += [(li, j0, split30) for j0 in range(0, OW, split30)]
        else:
            chunks.append((li, 0, OW))
    assert OW % tail == 0
    for j0 in range(0, OW, tail):
        chunks.append((ROWS_PER_CORE - 1, j0, tail))
    return chunks


def _out_layout(tail=10, split30=False):
    """Per-chunk group offsets in the ot tile: returns (chunks, group_off)
    where group_off[ci] is the first group index of chunk ci."""
    chunks = _schedule(tail, split30)
    off, group_off = 0, []
    for (li, j0, cp) in chunks:
        group_off.append(off)
        off += len(_chunk_groups(cp))
    return chunks, group_off, off


def _build_bass(reps=1, with_wdma=True, with_mm=True, with_out=True,
                tail=5, wbufs=8, alt_ring=True, wdt="e3m4", fillers=0,
                split30=5, rot_ring=False):
    import concourse.bass as bass
    import concourse.mybir as mybir
    import concourse.tile as tile
    from concourse import bacc

    chunks, group_off, n_groups = _out_layout(tail, split30)
    otw = n_groups * O
    cpmax = max(cp for _, _, cp in chunks)
    ppbufs = 7 if fillers else 8         # leave one PSUM bank for the scratch

    f32 = mybir.dt.float32
    bf16 = mybir.dt.bfloat16
    w_dt = {"bf16": bf16, "e3m4": mybir.dt.float8e3}[wdt]
    nc = bacc.Bacc("TRN2", target_bir_lowering=False, debug=False,
                   num_devices=NCORES)
    xsd = nc.dram_tensor("xs", [C, XROWS, W, B], bf16, kind="ExternalInput")
    wsd = nc.dram_tensor("ws", [C, P_CORE, 9, O], w_dt, kind="ExternalInput")
    od = nc.dram_tensor("out", [128, otw], bf16, kind="ExternalOutput")

    with tile.TileContext(nc) as tc:
        with (
            tc.tile_pool(name="xp", bufs=1) as xp,
            tc.tile_pool(name="wp", bufs=wbufs) as wp,
            tc.tile_pool(name="op", bufs=1) as op,
            tc.tile_pool(name="pp", bufs=ppbufs, space=bass.MemorySpace.PSUM) as pp,
            tc.tile_pool(name="fp", bufs=1, space=bass.MemorySpace.PSUM) as fp,
        ):
            xt = xp.tile([C, XROWS, W, B], bf16)
            # x rides the SWDGE ring so the HWDGE ring(s) are dedicated to
            # the weight stream (the critical path)
            nc.gpsimd.dma_start(xt[:], xsd[:])
            ot = op.tile([128, otw], bf16)

            wt_fixed = None
            if not with_wdma:
                # mm-only probe: one persistent weight tile, loaded once
                wt_fixed = xp.tile([C, cpmax, 9, O], w_dt)
                nc.sync.dma_start(wt_fixed[:], wsd[:, 0:cpmax, :, :])
            if not with_mm and with_out:
                nc.vector.memset(ot[:], 0.0)

            for _rep in range(reps):
                p0 = 0
                for ci, (li, j0, cp) in enumerate(chunks):
                    last = ci == len(chunks) - 1
                    if with_wdma:
                        wt = wp.tile([C, cp, 9, O], w_dt)
                        src = wsd[:, p0:p0 + cp, :, :]
                        if rot_ring:
                            eng = (nc.sync, nc.scalar,
                                   nc.gpsimd)[ci % 3]
                            eng.dma_start(wt[:], src)
                        elif alt_ring and ci % 2 == 1:
                            nc.scalar.dma_start(wt[:], src)
                        else:
                            nc.sync.dma_start(wt[:], src)
                    else:
                        wt = wt_fixed
                    if with_mm:
                        for gi, (jo, gsz) in enumerate(_chunk_groups(cp)):
                            jg = j0 + jo
                            ps = pp.tile([128, O], f32)
                            for dd in range(9):
                                di, dj = dd // 3, dd % 3
                                for u in range(gsz):
                                    nc.tensor.matmul(
                                        ps[32 * u:32 * (u + 1), :],
                                        xt[:, li + di, jg + u + dj, :],
                                        wt[:, jo + u, dd, :],
                                        start=(dd == 0),
                                        stop=(dd == 8),
                                        tile_position=(0, 32 * u),
                                    )
                            g = group_off[ci] + gi
                            nc.vector.tensor_copy(
                                ot[0:32 * gsz, g * O:(g + 1) * O],
                                ps[0:32 * gsz, :])
                    if with_mm and fillers and cp == OW and not last:
                        # keep the PE HAM activity window open across the
                        # DMA wait for the next chunk: dummy matmuls into a
                        # scratch bank, data-dependent on this chunk's ot
                        # region so they cannot be hoisted earlier
                        ge = (group_off[ci] + len(_chunk_groups(cp))) * O
                        sc = fp.tile([32, 512], f32)
                        for _k in range(fillers):
                            nc.tensor.matmul(
                                sc[:, :],
                                ot[0:C, ge - 32:ge],
                                ot[0:C, ge - 512:ge],
                                start=True, stop=True,
                            )
                    if with_out:
                        g0, g1 = group_off[ci], group_off[ci] + len(
                            _chunk_groups(cp))
                        dst = od[:, g0 * O:g1 * O]
                        srco = ot[:, g0 * O:g1 * O]
                        if last:
                            nc.sync.dma_start(dst, srco)
                        else:
                            nc.gpsimd.dma_start(dst, srco)
                    p0 += cp
    nc.compile()
    return nc


def _get_nc():
    key = tuple(sorted(KERNEL_KW.items()))
    if key not in _NC_CACHE:
        _NC_CACHE[key] = _build_bass(**KERNEL_KW)
    return _NC_CACHE[key]


def _prep_in_maps(x, weight, wdt="e3m4"):
    import ml_dtypes
    bf16 = ml_dtypes.bfloat16

    if wdt == "e3m4":
        wq = (weight.astype(np.float32) * WSCALE).astype(ml_dtypes.float8_e3m4)
    else:
        wq = weight.astype(bf16)
    # weight [900, O, C*3*3] -> [C, P_pad=960, dd, O]
    w5 = wq.reshape(OH * OW, O, C, 3, 3)
    wt = w5.transpose(2, 0, 3, 4, 1).reshape(C, OH * OW, 9, O)
    wpad = np.zeros((C, NCORES * P_CORE, 9, O), dtype=wq.dtype)
    wpad[:, :OH * OW] = wt

    # x [B, C, H, W] -> bf16 [C, H_pad=34, W, B]
    xt = x.astype(bf16).transpose(1, 2, 3, 0)
    xpad = np.zeros((C, H + 2, W, B), dtype=bf16)
    xpad[:, :H] = xt

    in_maps = []
    for c in range(NCORES):
        in_maps.append({
            "xs": np.ascontiguousarray(
                xpad[:, ROWS_PER_CORE * c:ROWS_PER_CORE * c + XROWS]),
            "ws": np.ascontiguousarray(
                wpad[:, P_CORE * c:P_CORE * (c + 1)]),
        })
    return in_maps


def kernel(x, weight, bias):
    global LAST_RESULT
    from concourse.bass_utils import run_bass_kernel_spmd

    x = np.asarray(x, dtype=np.float32)
    weight = np.asarray(weight, dtype=np.float32)
    bias = np.asarray(bias, dtype=np.float32)

    wdt = KERNEL_KW.get("wdt", "e3m4")
    in_maps = _prep_in_maps(x, weight, wdt=wdt)
    nc = _get_nc()
    LAST_RESULT = run_bass_kernel_spmd(
        nc, in_maps, core_ids=list(range(NCORES)), trace=False)

    # ---- gather: per-core [128, n_groups*96] -> full [B, O, 30, 30] ----
    tail = KERNEL_KW.get("tail", 5)
    chunks, group_off, n_groups = _out_layout(
        tail, KERNEL_KW.get("split30", 5))
    out = np.zeros((B, O, OH, OW), dtype=np.float32)
    for c in range(NCORES):
        oc = LAST_RESULT.results[c]["out"].astype(np.float32)
        if wdt == "e3m4":
            oc = oc / WSCALE
        oc = oc.reshape(4, 32, n_groups, O)
        for ci, (li, j0, cp) in enumerate(chunks):
            i = ROWS_PER_CORE * c + li
            if i >= OH:
                continue
            for gi, (jo, gsz) in enumerate(_chunk_groups(cp)):
                g = group_off[ci] + gi
                blk = oc[:gsz, :, g, :]            # [u, b, o]
                out[:, :, i, j0 + jo:j0 + jo + gsz] = blk.transpose(1, 2, 0)
    out += bias.reshape(OH, OW, O).transpose(2, 0, 1)[None]
    return out
